# revision 52
# baseline (speedup 1.0000x reference)
"""Trainium2 Bass kernel for AudioPreprocessingLayer.

Computes: floor(log2(mel_fb @ (rfft(x*hamming, norm=forward).real ** 2)))
for x of shape (4096, 32, 512), sharded batch-wise across 8 NeuronCores.

Key ideas:
  - rfft(.).real is a matmul with the cosine matrix C[n,k] = cos(2*pi*k*n/512)/512.
    The hamming window folds into it host-side: W = diag(hw) @ C, stored bf16.
  - Mel filterbank column 0 (DC bin) is structurally zero, so only bins 1..256
    are computed -> 256 = 2x128 clean chunks (checked at runtime, with a
    257-bin fallback).
  - x is cast once to bf16 by the load DMA; the on-chip transpose runs as
    REGULAR bf16 matmuls against an identity (1 cycle/row AND counts as PE
    activity, keeping the HAM clock gate at 2.4 GHz), then PSUM -> SBUF bf16
    copies split across the vector and scalar engines.
  - All loads stream on the gpsimd SWDGE queue with 7 macros of SBUF
    lookahead, so HBM never idles; all output stores are issued at the very
    end (from a persistent accumulator tile) so no store-semaphore wait ever
    blocks a compute queue.
  - floor(log2(m)) for positive fp32 m is exactly
    max(bitcast_int32(m) >> 23, 75) - 127   (the max() also maps the
    mels==0 -> eps=2^-52 case to -52 exactly).
  - Rows are mapped to partitions in blocks of JT per macro-group
    (row = m0 + JT*p + j), so every load DMA descriptor covers JT consecutive
    DRAM rows (16 KB in), matching one store DMA per macro.
"""

import os
import sys

for _p in ("/opt/trn_rl_repo",):
    if _p not in sys.path and os.path.isdir(_p):
        sys.path.append(_p)

import numpy as np
import ml_dtypes

import concourse.bass as bass
from concourse.bass import broadcast_tensor_aps
from concourse import bacc, mybir
from concourse.tile import TileContext
from concourse.bass_utils import run_bass_kernel_spmd
from concourse.masks import make_identity

N_CORES = 8
B, T, FRAME = 4096, 32, 512
R_PER_CORE = (B // N_CORES) * T  # 16384 rows of length 512 per core
N_MELS = 20

f32 = mybir.dt.float32
bf16 = mybir.dt.bfloat16
i32 = mybir.dt.int32


def _ceil_div(a, b):
    return (a + b - 1) // b


def build_graph(R=R_PER_CORE, NF=256, group_r=512, xbar_mod=0, plan="2mb"):
    """Build the SPMD Bass graph for one core's shard.

    x:   [R, 512]  f32   rows to transform
    w:   [4, 128, NF] bf16  cosine*window matrix, chunked along n
    fbt: [NFC, 128, N_MELS] bf16  mel filterbank transposed+chunked along freq
    out: [R, N_MELS] f32
    """
    assert R % group_r == 0 and group_r % 128 == 0
    RT = group_r // 128          # row subtiles per group (4)
    NQ = FRAME // 128            # 4 n-chunks
    NFC = _ceil_div(NF, 128)     # freq chunks
    f_sizes = [min(128, NF - 128 * c) for c in range(NFC)]

    nc = bacc.Bacc(None, target_bir_lowering=False)
    x_d = nc.declare_dram_parameter("x", [R, FRAME], f32, isOutput=False)
    w_d = nc.declare_dram_parameter("w", [NQ, 128, NF], bf16, isOutput=False)
    fbt_d = nc.declare_dram_parameter("fbt", [NFC, 128, N_MELS], bf16, isOutput=False)
    out_d = nc.declare_dram_parameter("out", [R, N_MELS], f32, isOutput=True)

    with TileContext(nc) as tc:
        with (
            tc.tile_pool(name="consts", bufs=1) as consts,
            tc.tile_pool(name="xb", bufs=13) as xb_pool,
            tc.tile_pool(name="xb32", bufs=2) as xb32_pool,
            tc.tile_pool(name="xt", bufs=3) as xt_pool,
            tc.tile_pool(name="xq", bufs=3) as xq_pool,
            tc.tile_pool(name="mag", bufs=4) as mag_pool,
            tc.tile_pool(name="fin", bufs=3) as fin_pool,
            tc.tile_pool(name="ps_xt", bufs=3, space="PSUM") as ps_xt_pool,
            tc.tile_pool(name="ps_y", bufs=2, space="PSUM") as ps_y_pool,
            tc.tile_pool(name="ps_m", bufs=1, space="PSUM") as ps_m_pool,
        ):
            # ---- constants (bf16 straight from DRAM, no casts) ----
            # per-partition constants, broadcast along free dims so the
            # finalize DVE ops are tensor_tensor-class (single-port mode;
            # 2-port DVE ops contend with the gpsimd SWDGE descriptor
            # generator that feeds the casting load DMAs)
            ones_c = consts.tile([128, 1], f32)
            nc.vector.memset(ones_c, 1.0)
            c23 = consts.tile([128, 1], i32)
            nc.vector.memset(c23, 23)
            c127 = consts.tile([128, 1], f32)
            nc.vector.memset(c127, 127.0)
            w_sb = consts.tile([128, NQ, NF], bf16)
            nc.sync.dma_start(out=w_sb, in_=w_d.rearrange("q p f -> p q f"))
            fbt_sb = consts.tile([128, NFC, N_MELS], bf16)
            nc.sync.dma_start(out=fbt_sb, in_=fbt_d.rearrange("c p m -> p c m"))
            # full per-core output staged in SBUF; stored at the very end
            o_all = consts.tile([128, (R // 128) * N_MELS], f32)

            # macro sizes (in groups): tiny head for fast ramp, 2-group body
            # (bigger macros speed the SWDGE ring but the heavier concurrent
            # DMA stream slows every SBUF engine port ~20% — 2 MB balances),
            # tiny tail to shrink the drain
            n_groups = R // group_r
            if n_groups == 32 and plan == "3mb":
                gpm_list = [1, 1] + [3] * 9 + [1, 1, 1]
            elif n_groups == 32 and plan == "2mb_tail":
                gpm_list = [1, 1] + [2] * 13 + [1, 1, 1, 1]
            elif n_groups >= 8:
                gpm_list = [1, 1] + [2] * ((n_groups - 4) // 2) + [1, 1]
            else:
                gpm_list = [1] * n_groups
            assert sum(gpm_list) == n_groups, (gpm_list, n_groups)

            groups = []   # (macro, gg) per group
            macros = []   # per macro: dict(m0, GPM, JT, off)
            # a couple of mid-stream macros load as f32 over the sync HWDGE
            # queue (casting DMAs are SWDGE-only) and get cast to bf16 on
            # vector+scalar: this runs in parallel with the gpsimd SWDGE
            # stream, taking total load time below the single-queue rate
            hw_set = {5, 10} if plan == "2mb_hw" and len(gpm_list) == 17 else set()
            m0 = 0
            for mg, GPM in enumerate(gpm_list):
                use_xbar = bool(xbar_mod) and (mg % xbar_mod == xbar_mod - 1)
                macros.append(
                    {"m0": m0, "GPM": GPM, "JT": GPM * RT,
                     "off": (m0 // 128) * N_MELS, "xbar": use_xbar,
                     "hwdge": mg in hw_set}
                )
                for gg in range(GPM):
                    groups.append((mg, gg))
                m0 += GPM * group_r

            def load_macro(mac):
                JT = mac["JT"]
                # casting DMA (f32 dram -> bf16 sbuf); row m0 + JT*p + j
                # -> partition p. Explicit 2D "(p) (j n)" AP: bass lowers
                # DMA APs with opt=False, so this emits one 16 KB
                # descriptor per partition instead of one per 2 KB row
                xb_sb = xb_pool.tile([128, JT * FRAME], bf16, name="xb_sb")
                x_ap = x_d[
                    mac["m0"] : mac["m0"] + JT * 128, :
                ].rearrange("(p j) n -> p (j n)", j=JT)
                if mac["hwdge"]:
                    xb32 = xb32_pool.tile([128, JT * FRAME], f32, name="xb32")
                    nc.sync.dma_start(out=xb32, in_=x_ap)
                    h = JT * FRAME // 2
                    nc.vector.tensor_copy(xb_sb[:, :h], xb32[:, :h])
                    nc.scalar.copy(xb_sb[:, h:], xb32[:, h:])
                else:
                    nc.gpsimd.dma_start(out=xb_sb, in_=x_ap)
                mac["xb"] = xb_sb

            # first two macro loads go out before make_identity's gpsimd
            # memsets, so HBM starts streaming ~1 us earlier
            load_macro(macros[0])
            load_macro(macros[1])
            ident = consts.tile([128, 128], bf16)
            make_identity(nc, ident)

            st = {}
            cp_engines = [nc.vector, nc.scalar, nc.vector]

            def stage_T(g):
                mg, gg = groups[g]
                mac = macros[mg]
                if gg == 0 and "xb" not in mac:
                    load_macro(mac)
                    if mac["xbar"]:
                        # whole-macro transpose on the DMA xbar (sync queue):
                        # xt[n', (j, q), p] with n = q*128 + n'
                        JT = mac["JT"]
                        xt_sb = xt_pool.tile(
                            [128, JT, NQ, 128], bf16, name="xt_sb"
                        )
                        nc.sync.dma_start(out=xt_sb, in_=xb_sb, transpose=True)
                        mac["xt"] = xt_sb
                if mac["xbar"]:
                    return
                # PE transpose of this group's RT row-blocks; PSUM -> SBUF
                # bf16 copies (exact: x is bf16-valued) round-robin 2:1
                # vector:scalar
                xb_sb = mac["xb"]
                xq_sb = []
                for q in range(NQ):
                    t = ps_xt_pool.tile(
                        [128, group_r], f32, name=f"xt{q}", tag="xt"
                    )
                    for j in range(RT):
                        base = (gg * RT + j) * FRAME + q * 128
                        nc.tensor.matmul(
                            t[:, j * 128 : (j + 1) * 128],
                            xb_sb[:, base : base + 128],
                            ident,
                            start=True,
                            stop=True,
                        )
                    dst = xq_pool.tile(
                        [128, group_r], bf16, name=f"xq{q}", tag=f"xq{q}"
                    )
                    xq_sb.append(dst)
                    eng = cp_engines[(g * NQ + q) % 3]
                    if eng is nc.scalar:
                        eng.copy(dst, t)
                    elif plan == "2mb_cast":
                        eng.tensor_copy(dst, t)
                    else:
                        # mult-by-1 tensor_tensor: exact, and single-port DVE
                        # mode — a 2-port CAST copy here starves the SWDGE
                        # descriptor generator and throttles the load stream
                        # (interleaved A/B: ~9 us faster than tensor_copy)
                        t_ap = t[:, :]
                        _, ones_b = broadcast_tensor_aps(t_ap, ones_c[:, 0:1])
                        eng.tensor_tensor(dst, t_ap, ones_b, mybir.AluOpType.mult)
                st[("xq", g)] = xq_sb

            def stage_M1(g):
                # matmul 1: yT[f, r] += W[n, f].T @ xT[n, r]
                mg, gg = groups[g]
                mac = macros[mg]
                y_ps = ps_y_pool.tile([128, NFC, group_r], f32, name="y_ps")
                if mac["xbar"]:
                    xt_sb = mac["xt"]
                    movings = [
                        xt_sb[:, gg * RT : (gg + 1) * RT, q, :]
                        for q in range(NQ)
                    ]
                else:
                    movings = st.pop(("xq", g))
                # c innermost: consecutive matmuls alternate PSUM banks
                for q in range(NQ):
                    for c in range(NFC):
                        fs = f_sizes[c]
                        nc.tensor.matmul(
                            y_ps[:fs, c, :],
                            w_sb[:, q, 128 * c : 128 * c + fs],
                            movings[q],
                            start=(q == 0),
                            stop=(q == NQ - 1),
                        )
                # square: magT = yT*yT (fused, psum -> sbuf bf16)
                mag_sb = mag_pool.tile([128, NFC, group_r], bf16, name="mag_sb")
                nc.scalar.activation(
                    mag_sb, y_ps, mybir.ActivationFunctionType.Square
                )
                st[("mag", g)] = mag_sb

            def stage_M2(g):
                mg, gg = groups[g]
                mac = macros[mg]
                mag_sb = st.pop(("mag", g))
                # matmul 2: mels[r, m] += magT[f, r].T @ fbt[f, m]
                mels_ps = ps_m_pool.tile([128, RT * N_MELS], f32, name="mels_ps")
                # j innermost reuses the fbt stationary; start=True only on
                # the first matmul into the bank (start clears has_written
                # for the WHOLE bank)
                for c in range(NFC):
                    fs = f_sizes[c]
                    for j in range(RT):
                        nc.tensor.matmul(
                            mels_ps[:, j * N_MELS : (j + 1) * N_MELS],
                            mag_sb[:fs, c, j * 128 : (j + 1) * 128],
                            fbt_sb[:fs, c, :],
                            start=(c == 0 and j == 0),
                            stop=(c == NFC - 1 and j == RT - 1),
                        )
                # finalize floor(log2(m)) = max(bits >> 23, 75) - 127 straight
                # into the persistent output tile; both ops tensor_tensor-class
                e_sb = fin_pool.tile([128, RT * N_MELS], i32, tag="e_sb", name="e_sb")
                m_ap = mels_ps.bitcast(i32)[:, :]
                _, c23_b = broadcast_tensor_aps(m_ap, c23[:, 0:1])
                nc.vector.tensor_tensor(
                    e_sb, m_ap, c23_b, mybir.AluOpType.logical_shift_right
                )
                o_off = mac["off"] + gg * RT * N_MELS
                e_ap = e_sb[:, :]
                _, c127_b = broadcast_tensor_aps(e_ap, c127[:, 0:1])
                nc.vector.scalar_tensor_tensor(
                    o_all[:, o_off : o_off + RT * N_MELS],
                    e_ap,
                    75.0,
                    c127_b,
                    mybir.AluOpType.max,
                    mybir.AluOpType.subtract,
                )

            # software pipeline on the PE queue: per step issue
            # [T(g), M1(g-1), M2(g-2)] so the copies of group g have a full
            # M1+M2 window of slack and Square(g-1) hides under T(g+1)+M1(g)
            NG_ALL = len(groups)
            if plan == "nopipe":
                for g in range(NG_ALL):
                    stage_T(g)
                    stage_M1(g)
                    stage_M2(g)
            else:
                for g in range(NG_ALL):
                    stage_T(g)
                    if g >= 1:
                        stage_M1(g - 1)
                    if g >= 2:
                        stage_M2(g - 2)
                stage_M1(NG_ALL - 1)
                stage_M2(NG_ALL - 2)
                stage_M2(NG_ALL - 1)

            # ---- all stores at the end: one DMA per macro ----
            for mac in macros:
                JT = mac["JT"]
                nc.sync.dma_start(
                    out=out_d[
                        mac["m0"] : mac["m0"] + JT * 128, :
                    ].rearrange("(p j) m -> p (j m)", j=JT),
                    in_=o_all[:, mac["off"] : mac["off"] + JT * N_MELS],
                )
    nc.compile()
    return nc


def _prep_weights(filter_banks, hw):
    """Host-side: cosine*window matrix (bf16) and transposed filterbank."""
    fb = np.asarray(filter_banks, dtype=np.float32)
    n_mels, n_bins = fb.shape  # (20, 257)
    assert n_mels == N_MELS and n_bins == FRAME // 2 + 1

    if np.all(fb[:, 0] == 0.0):
        k0 = 1  # DC bin unused by the filterbank (structurally true)
    else:
        k0 = 0
    NF = n_bins - k0

    n = np.arange(FRAME, dtype=np.float64)
    k = np.arange(k0, n_bins, dtype=np.float64)
    C = np.cos(2.0 * np.pi * np.outer(n, k) / FRAME) / FRAME
    W = (np.asarray(hw, dtype=np.float64)[:, None] * C).astype(ml_dtypes.bfloat16)
    NQ = FRAME // 128
    w_chunks = np.ascontiguousarray(W.reshape(NQ, 128, NF))

    NFC = _ceil_div(NF, 128)
    fbt = np.zeros((NFC, 128, N_MELS), dtype=ml_dtypes.bfloat16)
    fbT = fb[:, k0:].T.astype(ml_dtypes.bfloat16)  # [NF, 20]
    for c in range(NFC):
        fs = min(128, NF - 128 * c)
        fbt[c, :fs, :] = fbT[128 * c : 128 * c + fs, :]
    return w_chunks, fbt, NF


_CACHE = {}


def _get_graph(R, NF, group_r, xbar_mod, plan):
    key = (R, NF, group_r, xbar_mod, plan)
    if key not in _CACHE:
        _CACHE[key] = build_graph(R, NF, group_r, xbar_mod, plan)
    return _CACHE[key]


def kernel(inputs, filter_banks, hw, _trace=False, _group_r=512, _xbar_mod=0,
           _plan="2mb"):
    x = np.ascontiguousarray(np.asarray(inputs, dtype=np.float32))
    assert x.shape == (B, T, FRAME), x.shape
    w_chunks, fbt, NF = _prep_weights(filter_banks, hw)

    shards = x.reshape(N_CORES, B // N_CORES * T, FRAME)
    nc = _get_graph(R_PER_CORE, NF, _group_r, _xbar_mod, _plan)
    in_maps = [
        {"x": shards[i], "w": w_chunks, "fbt": fbt} for i in range(N_CORES)
    ]
    res = run_bass_kernel_spmd(
        nc, in_maps, core_ids=list(range(N_CORES)), trace=_trace
    )
    out = np.stack([res.results[i]["out"] for i in range(N_CORES)], axis=0)
    out = out.reshape(B, T, N_MELS, 1).astype(np.float32)
    if _trace:
        kernel._last_result = res
    return out


# revision 53
# speedup vs baseline: 1.2400x; 1.2400x over previous
"""Trainium2 Bass kernel for AudioPreprocessingLayer.

Computes: floor(log2(mel_fb @ (rfft(x*hamming, norm=forward).real ** 2)))
for x of shape (4096, 32, 512), sharded batch-wise across 8 NeuronCores.

Key ideas:
  - rfft(.).real is a matmul with the cosine matrix C[n,k] = cos(2*pi*k*n/512)/512.
    The hamming window folds into it host-side: W = diag(hw) @ C, stored bf16.
  - Mel filterbank column 0 (DC bin) is structurally zero, so only bins 1..256
    are computed -> 256 = 2x128 clean chunks (checked at runtime, with a
    257-bin fallback).
  - x is cast once to bf16 by the load DMA; the on-chip transpose runs as
    REGULAR bf16 matmuls against an identity (1 cycle/row AND counts as PE
    activity, keeping the HAM clock gate at 2.4 GHz), then PSUM -> SBUF bf16
    copies split across the vector and scalar engines.
  - All loads stream on the gpsimd SWDGE queue with 7 macros of SBUF
    lookahead, so HBM never idles; all output stores are issued at the very
    end (from a persistent accumulator tile) so no store-semaphore wait ever
    blocks a compute queue.
  - floor(log2(m)) for positive fp32 m is exactly
    max(bitcast_int32(m) >> 23, 75) - 127   (the max() also maps the
    mels==0 -> eps=2^-52 case to -52 exactly).
  - Rows are mapped to partitions in blocks of JT per macro-group
    (row = m0 + JT*p + j), so every load DMA descriptor covers JT consecutive
    DRAM rows (16 KB in), matching one store DMA per macro.
"""

import os
import sys

for _p in ("/opt/trn_rl_repo",):
    if _p not in sys.path and os.path.isdir(_p):
        sys.path.append(_p)

import numpy as np
import ml_dtypes

import concourse.bass as bass
from concourse.bass import broadcast_tensor_aps
from concourse import bacc, mybir
from concourse.tile import TileContext
from concourse.bass_utils import run_bass_kernel_spmd
from concourse.masks import make_identity

N_CORES = 8
B, T, FRAME = 4096, 32, 512
R_PER_CORE = (B // N_CORES) * T  # 16384 rows of length 512 per core
N_MELS = 20

f32 = mybir.dt.float32
bf16 = mybir.dt.bfloat16
i32 = mybir.dt.int32


def _ceil_div(a, b):
    return (a + b - 1) // b


def build_graph(R=R_PER_CORE, NF=256, group_r=512, xbar_mod=0, plan="2mb"):
    """Build the SPMD Bass graph for one core's shard.

    x:   [R, 512]  f32   rows to transform
    w:   [4, 128, NF] bf16  cosine*window matrix, chunked along n
    fbt: [NFC, 128, N_MELS] bf16  mel filterbank transposed+chunked along freq
    out: [R, N_MELS] f32
    """
    assert R % group_r == 0 and group_r % 128 == 0
    RT = group_r // 128          # row subtiles per group (4)
    NQ = FRAME // 128            # 4 n-chunks
    NFC = _ceil_div(NF, 128)     # freq chunks
    f_sizes = [min(128, NF - 128 * c) for c in range(NFC)]

    nc = bacc.Bacc(None, target_bir_lowering=False)
    x_d = nc.declare_dram_parameter("x", [R, FRAME], f32, isOutput=False)
    w_d = nc.declare_dram_parameter("w", [NQ, 128, NF], bf16, isOutput=False)
    fbt_d = nc.declare_dram_parameter("fbt", [NFC, 128, N_MELS], bf16, isOutput=False)
    out_d = nc.declare_dram_parameter("out", [R, N_MELS], f32, isOutput=True)

    with TileContext(nc) as tc:
        with (
            tc.tile_pool(name="consts", bufs=1) as consts,
            tc.tile_pool(name="xb", bufs=13) as xb_pool,
            tc.tile_pool(name="xb32", bufs=2) as xb32_pool,
            tc.tile_pool(name="xt", bufs=3) as xt_pool,
            tc.tile_pool(name="xq", bufs=3) as xq_pool,
            tc.tile_pool(name="mag", bufs=4) as mag_pool,
            tc.tile_pool(name="fin", bufs=3) as fin_pool,
            tc.tile_pool(name="ps_xt", bufs=3, space="PSUM") as ps_xt_pool,
            tc.tile_pool(name="ps_y", bufs=2, space="PSUM") as ps_y_pool,
            tc.tile_pool(name="ps_m", bufs=1, space="PSUM") as ps_m_pool,
        ):
            # ---- constants (bf16 straight from DRAM, no casts) ----
            # per-partition constants, broadcast along free dims so the
            # finalize DVE ops are tensor_tensor-class (single-port mode;
            # 2-port DVE ops contend with the gpsimd SWDGE descriptor
            # generator that feeds the casting load DMAs)
            ones_c = consts.tile([128, 1], f32)
            nc.vector.memset(ones_c, 1.0)
            c23 = consts.tile([128, 1], i32)
            nc.vector.memset(c23, 23)
            c127 = consts.tile([128, 1], f32)
            nc.vector.memset(c127, 127.0)
            w_sb = consts.tile([128, NQ, NF], bf16)
            nc.sync.dma_start(out=w_sb, in_=w_d.rearrange("q p f -> p q f"))
            fbt_sb = consts.tile([128, NFC, N_MELS], bf16)
            nc.sync.dma_start(out=fbt_sb, in_=fbt_d.rearrange("c p m -> p c m"))
            # full per-core output staged in SBUF; stored at the very end
            o_all = consts.tile([128, (R // 128) * N_MELS], f32)

            # macro sizes (in groups): tiny head for fast ramp, 2-group body
            # (bigger macros speed the SWDGE ring but the heavier concurrent
            # DMA stream slows every SBUF engine port ~20% — 2 MB balances),
            # tiny tail to shrink the drain
            n_groups = R // group_r
            if n_groups == 32 and plan == "3mb":
                gpm_list = [1, 1] + [3] * 9 + [1, 1, 1]
            elif n_groups == 32 and plan == "2mb_tail":
                gpm_list = [1, 1] + [2] * 13 + [1, 1, 1, 1]
            elif n_groups >= 8:
                gpm_list = [1, 1] + [2] * ((n_groups - 4) // 2) + [1, 1]
            else:
                gpm_list = [1] * n_groups
            assert sum(gpm_list) == n_groups, (gpm_list, n_groups)

            groups = []   # (macro, gg) per group
            macros = []   # per macro: dict(m0, GPM, JT, off)
            # a couple of mid-stream macros load as f32 over the sync HWDGE
            # queue (casting DMAs are SWDGE-only) and get cast to bf16 on
            # vector+scalar: this runs in parallel with the gpsimd SWDGE
            # stream, taking total load time below the single-queue rate
            hw_set = {5, 10} if plan == "2mb_hw" and len(gpm_list) == 17 else set()
            m0 = 0
            for mg, GPM in enumerate(gpm_list):
                use_xbar = bool(xbar_mod) and (mg % xbar_mod == xbar_mod - 1)
                macros.append(
                    {"m0": m0, "GPM": GPM, "JT": GPM * RT,
                     "off": (m0 // 128) * N_MELS, "xbar": use_xbar,
                     "hwdge": mg in hw_set}
                )
                for gg in range(GPM):
                    groups.append((mg, gg))
                m0 += GPM * group_r

            def load_macro(mac):
                JT = mac["JT"]
                # casting DMA (f32 dram -> bf16 sbuf); row m0 + JT*p + j
                # -> partition p. Explicit 2D "(p) (j n)" AP: bass lowers
                # DMA APs with opt=False, so this emits one 16 KB
                # descriptor per partition instead of one per 2 KB row
                xb_sb = xb_pool.tile([128, JT * FRAME], bf16, name="xb_sb")
                x_ap = x_d[
                    mac["m0"] : mac["m0"] + JT * 128, :
                ].rearrange("(p j) n -> p (j n)", j=JT)
                if mac["hwdge"]:
                    xb32 = xb32_pool.tile([128, JT * FRAME], f32, name="xb32")
                    nc.sync.dma_start(out=xb32, in_=x_ap)
                    h = JT * FRAME // 2
                    nc.vector.tensor_copy(xb_sb[:, :h], xb32[:, :h])
                    nc.scalar.copy(xb_sb[:, h:], xb32[:, h:])
                else:
                    nc.gpsimd.dma_start(out=xb_sb, in_=x_ap)
                mac["xb"] = xb_sb

            # first two macro loads go out before make_identity's gpsimd
            # memsets, so HBM starts streaming ~1 us earlier
            load_macro(macros[0])
            load_macro(macros[1])
            ident = consts.tile([128, 128], bf16)
            make_identity(nc, ident)

            st = {}
            cp_engines = [nc.vector, nc.scalar, nc.vector]

            def stage_T(g):
                mg, gg = groups[g]
                mac = macros[mg]
                if gg == 0 and "xb" not in mac:
                    load_macro(mac)
                    if mac["xbar"]:
                        # whole-macro transpose on the DMA xbar (sync queue):
                        # xt[n', (j, q), p] with n = q*128 + n'
                        JT = mac["JT"]
                        xt_sb = xt_pool.tile(
                            [128, JT, NQ, 128], bf16, name="xt_sb"
                        )
                        nc.sync.dma_start(out=xt_sb, in_=xb_sb, transpose=True)
                        mac["xt"] = xt_sb
                if mac["xbar"]:
                    return
                # PE transpose of this group's RT row-blocks; PSUM -> SBUF
                # bf16 copies (exact: x is bf16-valued) round-robin 2:1
                # vector:scalar
                xb_sb = mac["xb"]
                xq_sb = []
                for q in range(NQ):
                    t = ps_xt_pool.tile(
                        [128, group_r], f32, name=f"xt{q}", tag="xt"
                    )
                    for j in range(RT):
                        base = (gg * RT + j) * FRAME + q * 128
                        nc.tensor.matmul(
                            t[:, j * 128 : (j + 1) * 128],
                            xb_sb[:, base : base + 128],
                            ident,
                            start=True,
                            stop=True,
                        )
                    dst = xq_pool.tile(
                        [128, group_r], bf16, name=f"xq{q}", tag=f"xq{q}"
                    )
                    xq_sb.append(dst)
                    if plan == "2mb_vcopy":
                        eng = nc.vector  # scalar runs only Squares
                    else:
                        eng = cp_engines[(g * NQ + q) % 3]
                    if eng is nc.scalar:
                        eng.copy(dst, t)
                    elif plan == "2mb_cast":
                        eng.tensor_copy(dst, t)
                    else:
                        # mult-by-1 tensor_tensor: exact, and single-port DVE
                        # mode — a 2-port CAST copy here starves the SWDGE
                        # descriptor generator and throttles the load stream
                        # (interleaved A/B: ~9 us faster than tensor_copy)
                        t_ap = t[:, :]
                        _, ones_b = broadcast_tensor_aps(t_ap, ones_c[:, 0:1])
                        eng.tensor_tensor(dst, t_ap, ones_b, mybir.AluOpType.mult)
                st[("xq", g)] = xq_sb

            def stage_M1(g):
                # matmul 1: yT[f, r] += W[n, f].T @ xT[n, r]
                mg, gg = groups[g]
                mac = macros[mg]
                y_ps = ps_y_pool.tile([128, NFC, group_r], f32, name="y_ps")
                if mac["xbar"]:
                    xt_sb = mac["xt"]
                    movings = [
                        xt_sb[:, gg * RT : (gg + 1) * RT, q, :]
                        for q in range(NQ)
                    ]
                else:
                    movings = st.pop(("xq", g))
                # c innermost: consecutive matmuls alternate PSUM banks
                for q in range(NQ):
                    for c in range(NFC):
                        fs = f_sizes[c]
                        nc.tensor.matmul(
                            y_ps[:fs, c, :],
                            w_sb[:, q, 128 * c : 128 * c + fs],
                            movings[q],
                            start=(q == 0),
                            stop=(q == NQ - 1),
                        )
                # square: magT = yT*yT (fused, psum -> sbuf bf16)
                mag_sb = mag_pool.tile([128, NFC, group_r], bf16, name="mag_sb")
                nc.scalar.activation(
                    mag_sb, y_ps, mybir.ActivationFunctionType.Square
                )
                st[("mag", g)] = mag_sb

            def stage_M2(g):
                mg, gg = groups[g]
                mac = macros[mg]
                mag_sb = st.pop(("mag", g))
                # matmul 2: mels[r, m] += magT[f, r].T @ fbt[f, m]
                mels_ps = ps_m_pool.tile([128, RT * N_MELS], f32, name="mels_ps")
                # j innermost reuses the fbt stationary; start=True only on
                # the first matmul into the bank (start clears has_written
                # for the WHOLE bank)
                for c in range(NFC):
                    fs = f_sizes[c]
                    for j in range(RT):
                        nc.tensor.matmul(
                            mels_ps[:, j * N_MELS : (j + 1) * N_MELS],
                            mag_sb[:fs, c, j * 128 : (j + 1) * 128],
                            fbt_sb[:fs, c, :],
                            start=(c == 0 and j == 0),
                            stop=(c == NFC - 1 and j == RT - 1),
                        )
                # finalize floor(log2(m)) = max(bits >> 23, 75) - 127 straight
                # into the persistent output tile; both ops tensor_tensor-class
                e_sb = fin_pool.tile([128, RT * N_MELS], i32, tag="e_sb", name="e_sb")
                m_ap = mels_ps.bitcast(i32)[:, :]
                _, c23_b = broadcast_tensor_aps(m_ap, c23[:, 0:1])
                nc.vector.tensor_tensor(
                    e_sb, m_ap, c23_b, mybir.AluOpType.logical_shift_right
                )
                o_off = mac["off"] + gg * RT * N_MELS
                e_ap = e_sb[:, :]
                _, c127_b = broadcast_tensor_aps(e_ap, c127[:, 0:1])
                nc.vector.scalar_tensor_tensor(
                    o_all[:, o_off : o_off + RT * N_MELS],
                    e_ap,
                    75.0,
                    c127_b,
                    mybir.AluOpType.max,
                    mybir.AluOpType.subtract,
                )

            # software pipeline on the PE queue: per step issue
            # [T(g), M1(g-1), M2(g-2)] so the copies of group g have a full
            # M1+M2 window of slack and Square(g-1) hides under T(g+1)+M1(g)
            NG_ALL = len(groups)
            if plan == "nopipe":
                for g in range(NG_ALL):
                    stage_T(g)
                    stage_M1(g)
                    stage_M2(g)
            else:
                for g in range(NG_ALL):
                    stage_T(g)
                    if g >= 1:
                        stage_M1(g - 1)
                    if g >= 2:
                        stage_M2(g - 2)
                stage_M1(NG_ALL - 1)
                stage_M2(NG_ALL - 2)
                stage_M2(NG_ALL - 1)

            # ---- all stores at the end: one DMA per macro ----
            for mac in macros:
                JT = mac["JT"]
                nc.sync.dma_start(
                    out=out_d[
                        mac["m0"] : mac["m0"] + JT * 128, :
                    ].rearrange("(p j) m -> p (j m)", j=JT),
                    in_=o_all[:, mac["off"] : mac["off"] + JT * N_MELS],
                )
    nc.compile()
    return nc


def _prep_weights(filter_banks, hw):
    """Host-side: cosine*window matrix (bf16) and transposed filterbank."""
    fb = np.asarray(filter_banks, dtype=np.float32)
    n_mels, n_bins = fb.shape  # (20, 257)
    assert n_mels == N_MELS and n_bins == FRAME // 2 + 1

    if np.all(fb[:, 0] == 0.0):
        k0 = 1  # DC bin unused by the filterbank (structurally true)
    else:
        k0 = 0
    NF = n_bins - k0

    n = np.arange(FRAME, dtype=np.float64)
    k = np.arange(k0, n_bins, dtype=np.float64)
    C = np.cos(2.0 * np.pi * np.outer(n, k) / FRAME) / FRAME
    W = (np.asarray(hw, dtype=np.float64)[:, None] * C).astype(ml_dtypes.bfloat16)
    NQ = FRAME // 128
    w_chunks = np.ascontiguousarray(W.reshape(NQ, 128, NF))

    NFC = _ceil_div(NF, 128)
    fbt = np.zeros((NFC, 128, N_MELS), dtype=ml_dtypes.bfloat16)
    fbT = fb[:, k0:].T.astype(ml_dtypes.bfloat16)  # [NF, 20]
    for c in range(NFC):
        fs = min(128, NF - 128 * c)
        fbt[c, :fs, :] = fbT[128 * c : 128 * c + fs, :]
    return w_chunks, fbt, NF


_CACHE = {}


def _get_graph(R, NF, group_r, xbar_mod, plan):
    key = (R, NF, group_r, xbar_mod, plan)
    if key not in _CACHE:
        _CACHE[key] = build_graph(R, NF, group_r, xbar_mod, plan)
    return _CACHE[key]


def kernel(inputs, filter_banks, hw, _trace=False, _group_r=512, _xbar_mod=0,
           _plan="2mb"):
    x = np.ascontiguousarray(np.asarray(inputs, dtype=np.float32))
    assert x.shape == (B, T, FRAME), x.shape
    w_chunks, fbt, NF = _prep_weights(filter_banks, hw)

    shards = x.reshape(N_CORES, B // N_CORES * T, FRAME)
    nc = _get_graph(R_PER_CORE, NF, _group_r, _xbar_mod, _plan)
    in_maps = [
        {"x": shards[i], "w": w_chunks, "fbt": fbt} for i in range(N_CORES)
    ]
    res = run_bass_kernel_spmd(
        nc, in_maps, core_ids=list(range(N_CORES)), trace=_trace
    )
    out = np.stack([res.results[i]["out"] for i in range(N_CORES)], axis=0)
    out = out.reshape(B, T, N_MELS, 1).astype(np.float32)
    if _trace:
        kernel._last_result = res
    return out
